# revision 30
# baseline (speedup 1.0000x reference)
"""GQA causal attention block (x @ Wq/Wk/Wv -> causal GQA attention -> @ Wo)
for Trainium2, SPMD over 8 NeuronCores.

Sharding: 4 batches x 2 query-shards. Core c handles batch c//2 and the
interleaved set of 128-row query tiles {s, s+2, s+4, ...} (s = c%2), which
balances the causal-attention triangle between the two shards of a batch.
Each core computes k/v projections for the full sequence (cheap), q/o
projections and attention only for its own query rows, and writes its own
output rows -- no collectives. The host scatters the per-core outputs back
into the full tensor.

The SPMD program is identical on all cores; per-shard differences (which
query rows, causal mask alignment) are carried entirely by the input data
(a gathered "xq" tensor and two host-provided mask tiles).
"""

import sys

for _p in ("/opt/trn_rl_repo", "/root/.axon_site/_ro/trn_rl_repo"):
    if _p not in sys.path:
        sys.path.append(_p)

import numpy as np
import ml_dtypes

import concourse.bacc as bacc
import concourse.tile as tile
import concourse.mybir as mybir
from concourse.bass_utils import run_bass_kernel_spmd
from concourse import bass_isa

F32 = mybir.dt.float32
F32R = mybir.dt.float32r
BF16 = mybir.dt.bfloat16
AF = mybir.ActivationFunctionType
NEG = -1.0e6  # additive mask for disallowed keys (pre-softmax-scale)


class Cfg:
    def __init__(self, T, E, H, KV, n_batch, n_shard, blk):
        self.T, self.E, self.H, self.KV = T, E, H, KV
        self.D = 128
        self.G = H // KV
        self.NE = E // 128           # contraction chunks for projections
        self.n_batch = n_batch
        self.n_shard = n_shard
        self.n_cores = n_batch * n_shard
        self.RQ = T // n_shard       # query rows per core
        self.NJ = self.RQ // 128     # local 128-row query tiles
        self.BLK = blk               # query block for q-proj/o-proj
        self.NB = self.RQ // blk
        self.JPB = blk // 128
        self.NTA = T // 512          # phase-A 512-row t-tiles
        self.HKV = KV * self.D       # k/v projection width
        self.scale = 1.0 / float(np.sqrt(self.D))


FULL = Cfg(T=2048, E=2048, H=16, KV=4, n_batch=4, n_shard=2, blk=512)


def build(cfg):
    c = cfg
    G4 = c.H // 4                    # number of 4-head kv groups
    nc = bacc.Bacc("TRN2", target_bir_lowering=False, debug=False,
                   num_devices=c.n_cores)

    x_d = nc.dram_tensor("x", [c.T, c.E], BF16, kind="ExternalInput").ap()
    xq_d = nc.dram_tensor("xq", [c.RQ, c.E], BF16, kind="ExternalInput").ap()
    wq_d = nc.dram_tensor("Wq", [c.E, c.H * c.D], BF16, kind="ExternalInput").ap()
    wk_d = nc.dram_tensor("Wk", [c.E, c.HKV], BF16, kind="ExternalInput").ap()
    wv_d = nc.dram_tensor("Wv", [c.E, c.HKV], BF16, kind="ExternalInput").ap()
    wo_d = nc.dram_tensor("Wo", [c.H * c.D, c.E], BF16, kind="ExternalInput").ap()
    mask_d = nc.dram_tensor("masks", [c.n_shard, 128, 512], F32,
                            kind="ExternalInput").ap()
    idb_d = nc.dram_tensor("identb", [128, 128], BF16, kind="ExternalInput").ap()
    onesb_d = nc.dram_tensor("onesb", [128, 128], BF16, kind="ExternalInput").ap()
    o_d = nc.dram_tensor("o", [c.RQ, c.E], F32, kind="ExternalOutput").ap()

    from contextlib import ExitStack
    with tile.TileContext(nc) as tc:
        with ExitStack() as _st:
            def pool(name, bufs, space="SBUF"):
                return _st.enter_context(
                    tc.tile_pool(name=name, bufs=bufs, space=space))
            constp = pool("const", 1)
            kvp = pool("kv", c.KV * c.NTA)
            vp = pool("vsb", c.T // 128)
            qtp = pool("qt", 5)
            ytp = pool("yt", G4 * c.JPB)
            xnp = pool("xn", 8)
            xtap = pool("xta", c.NE + 4)
            wtp = pool("wt", 6)
            wq6p = pool("wq6", 6)
            wo6p = pool("wo6", 8)
            smp = pool("sm", 8)
            ps5 = pool("ps5", 4, space="PSUM")
            ps2 = pool("ps2", 2, space="PSUM")
            ps1 = pool("ps1", 2, space="PSUM")
            # --- constants ---
            masks = []
            for i in range(c.n_shard):
                m = constp.tile([128, 512], F32, tag=f"mask{i}")
                nc.sync.dma_start(m[:], mask_d[i])
                masks.append(m)
            identb = constp.tile([128, 128], BF16, tag="identb")
            nc.sync.dma_start(identb[:], idb_d[:])
            onesb = constp.tile([128, 128], BF16, tag="onesb")
            nc.sync.dma_start(onesb[:], onesb_d[:])

            # persistent activations
            kT = [[kvp.tile([128, 512], BF16, tag="kT",
                            name=f"kT{i}_{t}") for t in range(c.NTA)]
                  for i in range(c.KV)]
            v_sb = [vp.tile([128, c.HKV], BF16, tag="v", name=f"v{i}")
                    for i in range(c.T // 128)]

            def transpose_in(dst_tiles, src_d, row0, nrows):
                """Transpose nrows x E from DRAM rows row0.. into dst_tiles
                (one [128, nrows] tile per 128-col e-chunk), batching 4
                transposes per PSUM bank with one wide copy out."""
                nsub = nrows // 128
                for qa in range(c.NE // 4):
                    xns = []
                    for i in range(nsub):
                        xn = xnp.tile([128, 512], BF16, tag="xn",
                                      name=f"xn{i}")
                        nc.sync.dma_start(
                            xn[:], src_d[row0 + i * 128:row0 + (i + 1) * 128,
                                         qa * 512:(qa + 1) * 512])
                        xns.append(xn)
                    for eh in range(4):
                        e = qa * 4 + eh
                        ptr = ps2.tile([128, 512], BF16, tag="tp")
                        for i in range(nsub):
                            nc.tensor.transpose(
                                ptr[:, i * 128:(i + 1) * 128],
                                xns[i][:, eh * 128:(eh + 1) * 128], identb[:])
                        nc.vector.tensor_copy(dst_tiles[e][:, :nrows],
                                              ptr[:, :nrows])

            # ---------------- Phase A: k/v projection over full T -----------
            for tt in range(c.NTA):
                xts = [xtap.tile([128, 512], BF16, tag="xta", name=f"xta{e}")
                       for e in range(c.NE)]
                transpose_in(xts, x_d, tt * 512, 512)
                # kT pass
                psk = [ps5.tile([128, 512], F32, tag="ps512", name=f"psk{i}")
                       for i in range(c.KV)]
                for e in range(c.NE):
                    wk_t = wtp.tile([128, c.HKV], BF16, tag="wk")
                    nc.gpsimd.dma_start(wk_t[:], wk_d[e * 128:(e + 1) * 128, :])
                    for h in range(c.KV):
                        nc.tensor.matmul(psk[h][:],
                                         wk_t[:, h * 128:(h + 1) * 128],
                                         xts[e][:],
                                         start=(e == 0), stop=(e == c.NE - 1))
                for h in range(c.KV):
                    nc.vector.tensor_copy(kT[h][tt][:], psk[h][:])
                # v pass
                psv = [ps5.tile([128, c.HKV], F32, tag="ps512", name=f"psv{i}")
                       for i in range(4)]
                for e in range(c.NE):
                    wv_t = wtp.tile([128, c.HKV], BF16, tag="wv")
                    nc.gpsimd.dma_start(wv_t[:], wv_d[e * 128:(e + 1) * 128, :])
                    for i in range(4):
                        nc.tensor.matmul(psv[i][:],
                                         xts[e][:, i * 128:(i + 1) * 128],
                                         wv_t[:],
                                         start=(e == 0), stop=(e == c.NE - 1))
                for i in range(4):
                    nc.vector.tensor_copy(v_sb[tt * 4 + i][:], psv[i][:])

            # ---------------- Phase B: per query block ----------------------
            for blk in range(c.NB):
                xqt = [xtap.tile([128, c.BLK], BF16, tag="xta", name=f"xta{e}")
                       for e in range(c.NE)]
                transpose_in(xqt, xq_d, blk * c.BLK, c.BLK)

                yT = [ytp.tile([128, 512], BF16, tag="yT", name=f"yT{i}")
                      for i in range(G4 * c.JPB)]

                for g in range(G4):          # kv group = heads 4g..4g+3
                    # q projection for this group
                    psq = [ps5.tile([128, c.BLK], F32, tag="ps512",
                                    name=f"psq{i}") for i in range(4)]
                    for e in range(c.NE):
                        wq_t = wq6p.tile([128, 512], BF16, tag="wq")
                        nc.sync.dma_start(
                            wq_t[:], wq_d[e * 128:(e + 1) * 128,
                                          g * 512:(g + 1) * 512])
                        for hh in range(4):
                            nc.tensor.matmul(
                                psq[hh][:],
                                wq_t[:, hh * 128:(hh + 1) * 128],
                                xqt[e][:],
                                start=(e == 0), stop=(e == c.NE - 1))
                    qTj = []
                    for jj in range(c.JPB):
                        q = qtp.tile([128, 512], BF16, tag="qT",
                                     name=f"qTj{jj}")
                        for hh in range(4):
                            nc.vector.tensor_copy(
                                q[:, hh * 128:(hh + 1) * 128],
                                psq[hh][:, jj * 128:(jj + 1) * 128])
                        qTj.append(q)

                    for jj in range(c.JPB):
                        j = blk * c.JPB + jj
                        nk = c.n_shard * (j + 1)
                        psy = ps1.tile([128, 512], F32, tag="yt")
                        psums = ps2.tile([128, 512], F32, tag="tp",
                                         name="psums")
                        for kk in range(nk):
                            sct = ps5.tile([128, 512], F32, tag="ps512")
                            nc.tensor.matmul(
                                sct[:],
                                kT[g][kk // 4][:, (kk % 4) * 128:
                                               (kk % 4 + 1) * 128],
                                qTj[jj][:],
                                start=True, stop=True)
                            mi = kk - (nk - c.n_shard)
                            if mi >= 0:
                                nc.vector.tensor_add(sct[:], sct[:],
                                                     masks[mi][:])
                            pbt = smp.tile([128, 512], BF16, tag="pT")
                            nc.scalar.activation(pbt[:], sct[:], AF.Exp,
                                                 scale=c.scale)
                            nc.tensor.matmul(
                                psums[:], onesb[:], pbt[:],
                                start=(kk == 0), stop=(kk == nk - 1))
                            nc.tensor.matmul(
                                psy[:],
                                v_sb[kk][:, g * 128:(g + 1) * 128],
                                pbt[:],
                                start=(kk == 0), stop=(kk == nk - 1))
                        bsb = smp.tile([128, 512], F32, tag="bsb")
                        nc.vector.reciprocal(bsb[:], psums[:])
                        nc.vector.tensor_mul(yT[g * c.JPB + jj][:], psy[:],
                                             bsb[:])

                # o projection for this block
                for et in range(c.E // 512):
                    for tpair in range(max(1, c.JPB // 2)):
                        nts = min(2, c.JPB)
                        pso = [ps1.tile([128, 512], F32, tag="yt",
                                        name=f"pso{i}") for i in range(nts)]
                        for h in range(c.H):
                            g, hh = divmod(h, 4)
                            wo_t = wo6p.tile([128, 512], BF16, tag="wo")
                            nc.gpsimd.dma_start(
                                wo_t[:], wo_d[h * 128:(h + 1) * 128,
                                              et * 512:(et + 1) * 512])
                            for ii in range(nts):
                                tsub = tpair * 2 + ii
                                nc.tensor.matmul(
                                    pso[ii][:],
                                    yT[g * c.JPB + tsub][:, hh * 128:
                                       (hh + 1) * 128],
                                    wo_t[:],
                                    start=(h == 0), stop=(h == c.H - 1))
                        for ii in range(nts):
                            tsub = tpair * 2 + ii
                            r0 = (blk * c.JPB + tsub) * 128
                            osb = wtp.tile([128, 512], F32, tag="osb")
                            nc.vector.tensor_copy(osb[:], pso[ii][:])
                            nc.sync.dma_start(o_d[r0:r0 + 128,
                                                  et * 512:(et + 1) * 512],
                                              osb[:])

    nc.compile()
    return nc


def make_masks(cfg, s):
    """Additive causal masks in scoresT ([key, query]) orientation, tiled
    4x along the free axis for the 4-head packing.

    For shard s, local q-tile j maps to global tile g = j*n_shard + s; the
    program adds mask[mi] to key subtile j*n_shard + mi of scoresT.
    mi < s: keep; mi == s: keep keys k <= q; mi > s: drop all.
    """
    r = np.arange(128)
    triT = np.where(r[:, None] <= r[None, :], 0.0, NEG).astype(np.float32)
    out = np.zeros((cfg.n_shard, 128, 128), np.float32)
    for mi in range(cfg.n_shard):
        if mi == s:
            out[mi] = triT
        elif mi > s:
            out[mi] = NEG
    return np.tile(out, (1, 1, 4))


def make_inputs(cfg, x, Wq, Wk, Wv, Wo):
    """Per-core input maps from full tensors (activations/weights in bf16)."""
    bf = ml_dtypes.bfloat16
    ident_b = np.eye(128, dtype=bf)
    Wqb, Wkb, Wvb, Wob = (w.astype(bf) for w in (Wq, Wk, Wv, Wo))
    in_maps = []
    for c in range(cfg.n_cores):
        b, s = divmod(c, cfg.n_shard)
        xb = np.ascontiguousarray(x[b].astype(bf))
        xq = np.ascontiguousarray(
            xb.reshape(cfg.T // 128, 128, cfg.E)[s::cfg.n_shard]
            .reshape(cfg.RQ, cfg.E))
        in_maps.append({
            "x": xb, "xq": xq, "Wq": Wqb, "Wk": Wkb, "Wv": Wvb, "Wo": Wob,
            "masks": make_masks(cfg, s),
            "identb": ident_b,
            "onesb": np.ones((128, 128), ml_dtypes.bfloat16),
        })
    return in_maps


def scatter_out(cfg, results):
    B = cfg.n_batch
    out = np.empty((B, cfg.T, cfg.E), np.float32)
    for c in range(cfg.n_cores):
        b, s = divmod(c, cfg.n_shard)
        out[b].reshape(cfg.T // 128, 128, cfg.E)[s::cfg.n_shard] = \
            results[c]["o"].reshape(cfg.RQ // 128, 128, cfg.E)
    return out


_NC_CACHE = {}


def get_nc(cfg):
    key = (cfg.T, cfg.E, cfg.H, cfg.KV, cfg.n_batch, cfg.n_shard, cfg.BLK)
    if key not in _NC_CACHE:
        _NC_CACHE[key] = build(cfg)
    return _NC_CACHE[key]


def run_on_hw(cfg, x, Wq, Wk, Wv, Wo, trace=False):
    nc = get_nc(cfg)
    in_maps = make_inputs(cfg, x, Wq, Wk, Wv, Wo)
    res = run_bass_kernel_spmd(nc, in_maps, list(range(cfg.n_cores)),
                               trace=trace)
    return scatter_out(cfg, [r for r in res.results]), res


def kernel(x, Wq, Wk, Wv, Wo):
    out, _ = run_on_hw(FULL, np.asarray(x), np.asarray(Wq), np.asarray(Wk),
                       np.asarray(Wv), np.asarray(Wo))
    return out


# revision 32
# speedup vs baseline: 1.1356x; 1.1356x over previous
"""GQA causal attention block (x @ Wq/Wk/Wv -> causal GQA attention -> @ Wo)
for Trainium2, SPMD over 8 NeuronCores.

Sharding: 4 batches x 2 query-shards. Core c handles batch c//2 and the
interleaved set of 128-row query tiles {s, s+2, s+4, ...} (s = c%2), which
balances the causal-attention triangle between the two shards of a batch.
Each core computes k/v projections for the full sequence (cheap), q/o
projections and attention only for its own query rows, and writes its own
output rows -- no collectives. The host scatters the per-core outputs back
into the full tensor.

The SPMD program is identical on all cores; per-shard differences (which
query rows, causal mask alignment) are carried entirely by the input data
(a gathered "xq" tensor and two host-provided mask tiles).
"""

import sys

for _p in ("/opt/trn_rl_repo", "/root/.axon_site/_ro/trn_rl_repo"):
    if _p not in sys.path:
        sys.path.append(_p)

import numpy as np
import ml_dtypes

import concourse.bacc as bacc
import concourse.tile as tile
import concourse.mybir as mybir
from concourse.bass_utils import run_bass_kernel_spmd
from concourse import bass_isa

F32 = mybir.dt.float32
F32R = mybir.dt.float32r
BF16 = mybir.dt.bfloat16
AF = mybir.ActivationFunctionType
NEG = -1.0e6  # additive mask for disallowed keys (pre-softmax-scale)


class Cfg:
    def __init__(self, T, E, H, KV, n_batch, n_shard, blk):
        self.T, self.E, self.H, self.KV = T, E, H, KV
        self.D = 128
        self.G = H // KV
        self.NE = E // 128           # contraction chunks for projections
        self.n_batch = n_batch
        self.n_shard = n_shard
        self.n_cores = n_batch * n_shard
        self.RQ = T // n_shard       # query rows per core
        self.NJ = self.RQ // 128     # local 128-row query tiles
        self.BLK = blk               # query block for q-proj/o-proj
        self.NB = self.RQ // blk
        self.JPB = blk // 128
        self.NTA = T // 512          # phase-A 512-row t-tiles
        self.HKV = KV * self.D       # k/v projection width
        self.scale = 1.0 / float(np.sqrt(self.D))


FULL = Cfg(T=2048, E=2048, H=16, KV=4, n_batch=4, n_shard=2, blk=512)


def build(cfg):
    c = cfg
    G4 = c.H // 4                    # number of 4-head kv groups
    nc = bacc.Bacc("TRN2", target_bir_lowering=False, debug=False,
                   num_devices=c.n_cores)

    x_d = nc.dram_tensor("x", [c.T, c.E], BF16, kind="ExternalInput").ap()
    xq_d = nc.dram_tensor("xq", [c.RQ, c.E], BF16, kind="ExternalInput").ap()
    wq_d = nc.dram_tensor("Wq", [c.E, c.H * c.D], BF16, kind="ExternalInput").ap()
    wk_d = nc.dram_tensor("Wk", [c.E, c.HKV], BF16, kind="ExternalInput").ap()
    wv_d = nc.dram_tensor("Wv", [c.E, c.HKV], BF16, kind="ExternalInput").ap()
    wo_d = nc.dram_tensor("Wo", [c.H * c.D, c.E], BF16, kind="ExternalInput").ap()
    mask_d = nc.dram_tensor("masks", [c.n_shard, 128, 512], F32,
                            kind="ExternalInput").ap()
    idb_d = nc.dram_tensor("identb", [128, 128], BF16, kind="ExternalInput").ap()
    onesb_d = nc.dram_tensor("onesb", [128, 128], BF16, kind="ExternalInput").ap()
    o_d = nc.dram_tensor("o", [c.RQ, c.E], F32, kind="ExternalOutput").ap()

    from contextlib import ExitStack
    with tile.TileContext(nc) as tc:
        with ExitStack() as _st:
            def pool(name, bufs, space="SBUF"):
                return _st.enter_context(
                    tc.tile_pool(name=name, bufs=bufs, space=space))
            constp = pool("const", 1)
            kvp = pool("kv", c.KV * c.NTA)
            vp = pool("vsb", c.T // 128)
            qtp = pool("qt", 5)
            ytp = pool("yt", G4 * c.JPB)
            xnp = pool("xn", 8)
            xtap = pool("xta", c.NE + 4)
            wtp = pool("wt", 6)
            wq6p = pool("wq6", 6)
            wo6p = pool("wo6", 8)
            smp = pool("sm", 8)
            ps5 = pool("ps5", 4, space="PSUM")
            ps2 = pool("ps2", 2, space="PSUM")
            ps1 = pool("ps1", 2, space="PSUM")
            # --- constants ---
            masks = []
            for i in range(c.n_shard):
                m = constp.tile([128, 512], F32, tag=f"mask{i}")
                nc.sync.dma_start(m[:], mask_d[i])
                masks.append(m)
            identb = constp.tile([128, 128], BF16, tag="identb")
            nc.sync.dma_start(identb[:], idb_d[:])
            onesb = constp.tile([128, 128], BF16, tag="onesb")
            nc.sync.dma_start(onesb[:], onesb_d[:])

            # warm the PE clock-gate during the initial DMA ramp
            pwu = ps2.tile([128, 512], BF16, tag="tp")
            for wu in range(24):
                nc.tensor.transpose(pwu[:, (wu % 4) * 128:(wu % 4 + 1) * 128],
                                    identb[:], identb[:])

            # persistent activations
            kT = [[kvp.tile([128, 512], BF16, tag="kT",
                            name=f"kT{i}_{t}") for t in range(c.NTA)]
                  for i in range(c.KV)]
            v_sb = [vp.tile([128, c.HKV], BF16, tag="v", name=f"v{i}")
                    for i in range(c.T // 128)]

            def transpose_in(dst_tiles, src_d, row0, nrows):
                """Transpose nrows x E from DRAM rows row0.. into dst_tiles
                (one [128, nrows] tile per 128-col e-chunk), batching 4
                transposes per PSUM bank with one wide copy out."""
                nsub = nrows // 128
                for qa in range(c.NE // 4):
                    xns = []
                    for i in range(nsub):
                        xn = xnp.tile([128, 512], BF16, tag="xn",
                                      name=f"xn{i}")
                        nc.sync.dma_start(
                            xn[:], src_d[row0 + i * 128:row0 + (i + 1) * 128,
                                         qa * 512:(qa + 1) * 512])
                        xns.append(xn)
                    for eh in range(4):
                        e = qa * 4 + eh
                        ptr = ps2.tile([128, 512], BF16, tag="tp")
                        for i in range(nsub):
                            nc.tensor.transpose(
                                ptr[:, i * 128:(i + 1) * 128],
                                xns[i][:, eh * 128:(eh + 1) * 128], identb[:])
                        nc.vector.tensor_copy(dst_tiles[e][:, :nrows],
                                              ptr[:, :nrows])

            # ---------------- Phase A: k/v projection over full T -----------
            for tt in range(c.NTA):
                xts = [xtap.tile([128, 512], BF16, tag="xta", name=f"xta{e}")
                       for e in range(c.NE)]
                transpose_in(xts, x_d, tt * 512, 512)
                # kT pass
                psk = [ps5.tile([128, 512], F32, tag="ps512", name=f"psk{i}")
                       for i in range(c.KV)]
                for e in range(c.NE):
                    wk_t = wtp.tile([128, c.HKV], BF16, tag="wk")
                    nc.gpsimd.dma_start(wk_t[:], wk_d[e * 128:(e + 1) * 128, :])
                    for h in range(c.KV):
                        nc.tensor.matmul(psk[h][:],
                                         wk_t[:, h * 128:(h + 1) * 128],
                                         xts[e][:],
                                         start=(e == 0), stop=(e == c.NE - 1))
                for h in range(c.KV):
                    nc.vector.tensor_copy(kT[h][tt][:], psk[h][:])
                # v pass
                psv = [ps5.tile([128, c.HKV], F32, tag="ps512", name=f"psv{i}")
                       for i in range(4)]
                for e in range(c.NE):
                    wv_t = wtp.tile([128, c.HKV], BF16, tag="wv")
                    nc.gpsimd.dma_start(wv_t[:], wv_d[e * 128:(e + 1) * 128, :])
                    for i in range(4):
                        nc.tensor.matmul(psv[i][:],
                                         xts[e][:, i * 128:(i + 1) * 128],
                                         wv_t[:],
                                         start=(e == 0), stop=(e == c.NE - 1))
                for i in range(4):
                    nc.vector.tensor_copy(v_sb[tt * 4 + i][:], psv[i][:])

            # ---------------- Phase B: per query block ----------------------
            for blk in range(c.NB):
                xqt = [xtap.tile([128, c.BLK], BF16, tag="xta", name=f"xta{e}")
                       for e in range(c.NE)]
                transpose_in(xqt, xq_d, blk * c.BLK, c.BLK)

                yT = [ytp.tile([128, 512], BF16, tag="yT", name=f"yT{i}")
                      for i in range(G4 * c.JPB)]

                for g in range(G4):          # kv group = heads 4g..4g+3
                    # q projection for this group
                    psq = [ps5.tile([128, c.BLK], F32, tag="ps512",
                                    name=f"psq{i}") for i in range(4)]
                    for e in range(c.NE):
                        wq_t = wq6p.tile([128, 512], BF16, tag="wq")
                        nc.sync.dma_start(
                            wq_t[:], wq_d[e * 128:(e + 1) * 128,
                                          g * 512:(g + 1) * 512])
                        for hh in range(4):
                            nc.tensor.matmul(
                                psq[hh][:],
                                wq_t[:, hh * 128:(hh + 1) * 128],
                                xqt[e][:],
                                start=(e == 0), stop=(e == c.NE - 1))
                    qTj = []
                    for jj in range(c.JPB):
                        q = qtp.tile([128, 512], BF16, tag="qT",
                                     name=f"qTj{jj}")
                        for hh in range(4):
                            nc.vector.tensor_copy(
                                q[:, hh * 128:(hh + 1) * 128],
                                psq[hh][:, jj * 128:(jj + 1) * 128])
                        qTj.append(q)

                    for jj in range(c.JPB):
                        j = blk * c.JPB + jj
                        nk = c.n_shard * (j + 1)
                        psy = ps1.tile([128, 512], F32, tag="yt")
                        psums = ps2.tile([128, 512], F32, tag="tp",
                                         name="psums")
                        for kk in range(nk):
                            sct = ps5.tile([128, 512], F32, tag="ps512")
                            nc.tensor.matmul(
                                sct[:],
                                kT[g][kk // 4][:, (kk % 4) * 128:
                                               (kk % 4 + 1) * 128],
                                qTj[jj][:],
                                start=True, stop=True)
                            mi = kk - (nk - c.n_shard)
                            if mi >= 0:
                                nc.vector.tensor_add(sct[:], sct[:],
                                                     masks[mi][:])
                            pbt = smp.tile([128, 512], BF16, tag="pT")
                            nc.scalar.activation(pbt[:], sct[:], AF.Exp,
                                                 scale=c.scale)
                            nc.tensor.matmul(
                                psums[:], onesb[:], pbt[:],
                                start=(kk == 0), stop=(kk == nk - 1))
                            nc.tensor.matmul(
                                psy[:],
                                v_sb[kk][:, g * 128:(g + 1) * 128],
                                pbt[:],
                                start=(kk == 0), stop=(kk == nk - 1))
                        bsb = smp.tile([128, 512], F32, tag="bsb")
                        nc.vector.reciprocal(bsb[:], psums[:])
                        nc.vector.tensor_mul(yT[g * c.JPB + jj][:], psy[:],
                                             bsb[:])

                # o projection for this block
                for et in range(c.E // 512):
                    pso = [ps5.tile([128, 512], F32, tag="ps512",
                                    name=f"pso{i}") for i in range(c.JPB)]
                    for h in range(c.H):
                        g, hh = divmod(h, 4)
                        wo_t = wo6p.tile([128, 512], BF16, tag="wo")
                        nc.gpsimd.dma_start(
                            wo_t[:], wo_d[h * 128:(h + 1) * 128,
                                          et * 512:(et + 1) * 512])
                        for tsub in range(c.JPB):
                            nc.tensor.matmul(
                                pso[tsub][:],
                                yT[g * c.JPB + tsub][:, hh * 128:(hh + 1) * 128],
                                wo_t[:],
                                start=(h == 0), stop=(h == c.H - 1))
                    for tsub in range(c.JPB):
                        r0 = (blk * c.JPB + tsub) * 128
                        osb = wtp.tile([128, 512], F32, tag="osb")
                        nc.vector.tensor_copy(osb[:], pso[tsub][:])
                        nc.sync.dma_start(o_d[r0:r0 + 128,
                                              et * 512:(et + 1) * 512],
                                          osb[:])

    nc.compile()
    return nc


def make_masks(cfg, s):
    """Additive causal masks in scoresT ([key, query]) orientation, tiled
    4x along the free axis for the 4-head packing.

    For shard s, local q-tile j maps to global tile g = j*n_shard + s; the
    program adds mask[mi] to key subtile j*n_shard + mi of scoresT.
    mi < s: keep; mi == s: keep keys k <= q; mi > s: drop all.
    """
    r = np.arange(128)
    triT = np.where(r[:, None] <= r[None, :], 0.0, NEG).astype(np.float32)
    out = np.zeros((cfg.n_shard, 128, 128), np.float32)
    for mi in range(cfg.n_shard):
        if mi == s:
            out[mi] = triT
        elif mi > s:
            out[mi] = NEG
    return np.tile(out, (1, 1, 4))


def make_inputs(cfg, x, Wq, Wk, Wv, Wo):
    """Per-core input maps from full tensors (activations/weights in bf16)."""
    bf = ml_dtypes.bfloat16
    ident_b = np.eye(128, dtype=bf)
    Wqb, Wkb, Wvb, Wob = (w.astype(bf) for w in (Wq, Wk, Wv, Wo))
    in_maps = []
    for c in range(cfg.n_cores):
        b, s = divmod(c, cfg.n_shard)
        xb = np.ascontiguousarray(x[b].astype(bf))
        xq = np.ascontiguousarray(
            xb.reshape(cfg.T // 128, 128, cfg.E)[s::cfg.n_shard]
            .reshape(cfg.RQ, cfg.E))
        in_maps.append({
            "x": xb, "xq": xq, "Wq": Wqb, "Wk": Wkb, "Wv": Wvb, "Wo": Wob,
            "masks": make_masks(cfg, s),
            "identb": ident_b,
            "onesb": np.ones((128, 128), ml_dtypes.bfloat16),
        })
    return in_maps


def scatter_out(cfg, results):
    B = cfg.n_batch
    out = np.empty((B, cfg.T, cfg.E), np.float32)
    for c in range(cfg.n_cores):
        b, s = divmod(c, cfg.n_shard)
        out[b].reshape(cfg.T // 128, 128, cfg.E)[s::cfg.n_shard] = \
            results[c]["o"].reshape(cfg.RQ // 128, 128, cfg.E)
    return out


_NC_CACHE = {}


def get_nc(cfg):
    key = (cfg.T, cfg.E, cfg.H, cfg.KV, cfg.n_batch, cfg.n_shard, cfg.BLK)
    if key not in _NC_CACHE:
        _NC_CACHE[key] = build(cfg)
    return _NC_CACHE[key]


def run_on_hw(cfg, x, Wq, Wk, Wv, Wo, trace=False):
    nc = get_nc(cfg)
    in_maps = make_inputs(cfg, x, Wq, Wk, Wv, Wo)
    res = run_bass_kernel_spmd(nc, in_maps, list(range(cfg.n_cores)),
                               trace=trace)
    return scatter_out(cfg, [r for r in res.results]), res


def kernel(x, Wq, Wk, Wv, Wo):
    out, _ = run_on_hw(FULL, np.asarray(x), np.asarray(Wq), np.asarray(Wk),
                       np.asarray(Wv), np.asarray(Wo))
    return out
